# revision 7
# baseline (speedup 1.0000x reference)
"""Trainium2 Bass kernel for nn_Decoder_74998718923424.

2-layer post-LN transformer decoder (self-attn + cross-attn + FFN),
B=2, T=1024, D=512, H=8, F=2048, V=32000, fp32 I/O.

Sharding: sequence-parallel over the 2048 (b, t) tokens -> 8 cores x 256
tokens; cores 0-3 own batch 0, cores 4-7 batch 1.  Every core holds the
full weights (streamed tile-wise from HBM, bf16) and computes its token
shard through the whole network.  The only cross-core exchanges are one
AllGather of self-attention K/V per layer within each 4-core group.
Cross-attention K/V (from the replicated encoder output x) are computed
redundantly per group.  Matmuls run in bf16 with fp32 PSUM accumulation;
the residual/LN stream is kept in fp32.
"""

import os
from contextlib import ExitStack

import numpy as np
import ml_dtypes

import concourse.bass as bass
import concourse.mybir as mybir
import concourse.tile as tile
from concourse import bacc
from concourse.bass_utils import run_bass_kernel_spmd
from concourse.masks import make_identity

B, T, D, H, F, L, V = 2, 1024, 512, 8, 2048, 2, 32000
HD = D // H           # 64
NC = 8                # cores
GP = 4                # cores per batch group
TPC = (B * T) // NC   # 256 tokens per core
KT = D // 128         # 4 k-tiles over D
FT = F // 128         # 16 tiles over F
NTK = T // 128        # 8 key tiles per attention
EPS = 1e-5

f32 = mybir.dt.float32
bf16 = mybir.dt.bfloat16
i32 = mybir.dt.int32
AF = mybir.ActivationFunctionType
BF = ml_dtypes.bfloat16

KV_A = D * TPC              # k^T shard elems  [512, 256]
KV_B = TPC * D              # v shard elems    [256, 512]
KVN = KV_A + KV_B           # per-rank AG payload elems
GROUPS = [[0, 1, 2, 3], [4, 5, 6, 7]]

_CACHE = {}


# --------------------------------------------------------------------------
# device kernel
# --------------------------------------------------------------------------

def _build_module():
    nc = bacc.Bacc("TRN2", target_bir_lowering=False, debug=False, num_devices=NC)

    def din(name, shape, dt=bf16):
        return nc.dram_tensor(name, shape, dt, kind="ExternalInput")

    tok_d = din("tok", [2, 128], i32)
    emb_d = din("emb16", [V, D])
    pos_d = din("posT", [D, TPC], f32)
    xt_d = din("xT16", [D, T])
    # lhsT-tiled weights: [l, m, p, kt, c] with w[kt*128+p, m*128+c]
    wqk_d = din("wqk", [L, 8, 128, KT, 128])      # Wqkv cols 0:1024 (q,k)
    wqkv_v_d = din("wqkv_v", [L, 128, KT, 512])   # Wqkv cols 1024:1536 rhs-tiled
    wo_d = din("wo", [L, 4, 128, KT, 128])
    wkv_k_d = din("wkv_k", [L, 4, 128, KT, 128])  # Wkv cols 0:512
    wkv_v_d = din("wkv_v", [L, 128, KT, 512])     # Wkv cols 512:1024 rhs-tiled
    wq_d = din("wq", [L, 4, 128, KT, 128])
    wco_d = din("wco", [L, 4, 128, KT, 128])
    w1_d = din("w1", [L, FT, 128, KT, 128])
    w2_d = din("w2", [L, 4, 128, FT, 128])
    bqkv_d = din("bqkv", [L, 3 * D], f32)
    bkv_d = din("bkv", [L, 2 * D], f32)
    bq_d = din("bq", [L, D], f32)
    bo_d = din("bo", [L, D], f32)
    bco_d = din("bco", [L, D], f32)
    b1_d = din("b1", [L, F], f32)
    b2_d = din("b2", [L, D], f32)
    lng_d = din("lng", [L, 3, D], f32)   # g1,g2,g3
    lnb_d = din("lnb", [L, 3, D], f32)   # be1,be2,be3

    out_d = nc.dram_tensor("yT_out", [D, TPC], f32, kind="ExternalOutput")

    kv_in = [nc.dram_tensor(f"kv_in{l}", [KVN], bf16) for l in range(L)]
    kv_out = [nc.dram_tensor(f"kv_out{l}", [GP * KVN], bf16) for l in range(L)]

    with tile.TileContext(nc) as tc, ExitStack() as ctx:
        pool = lambda name, bufs: ctx.enter_context(tc.tile_pool(name=name, bufs=bufs))
        cst = pool("cst", 1)
        biasp = pool("bias", 2)
        wt4 = pool("wt4", 6)
        wt16 = pool("wt16", 2)
        wvr = pool("wvr", 2)
        gthp = pool("gth", 2)
        yp = pool("y", 2)
        kvl = pool("kvl", 2)
        qp = pool("q", 2)
        kxp = pool("kx", 1)
        vxp = pool("vx", 1)
        vsp = pool("vs", 1)
        khp = pool("kh", 1)
        ptp = pool("pt", 2)
        otp = pool("ot", 2)
        htp = pool("ht", 1)
        cp = pool("c", 1)
        tmpp = pool("tmp", 2)
        smp = pool("sm", 2)
        ps256 = ctx.enter_context(tc.tile_pool(name="ps256", bufs=3, space="PSUM"))
        psw = ctx.enter_context(tc.tile_pool(name="psw", bufs=2, space="PSUM"))
        psav = ctx.enter_context(tc.tile_pool(name="psav", bufs=2, space="PSUM"))
        psst = ctx.enter_context(tc.tile_pool(name="psst", bufs=1, space="PSUM"))

        ident = cst.tile([128, 128], bf16, tag="ident")
        make_identity(nc, ident[:])
        ones16 = cst.tile([128, 1], bf16, tag="ones16")
        nc.vector.memset(ones16[:], 1.0)
        eps_sb = cst.tile([1, 1], f32, tag="eps")
        nc.vector.memset(eps_sb[:], EPS)

        posT = cst.tile([128, KT, TPC], f32, tag="posT")
        nc.sync.dma_start(out=posT[:], in_=pos_d.ap().rearrange("(kt p) t -> p kt t", p=128))
        xt16 = cst.tile([128, KT, T], bf16, tag="xt16")
        nc.sync.dma_start(out=xt16[:], in_=xt_d.ap().rearrange("(kt p) t -> p kt t", p=128))

        # ---------------- embedding gather + transpose ----------------
        idx = cst.tile([128, 2], i32, tag="idx")
        nc.sync.dma_start(out=idx[:], in_=tok_d.ap().rearrange("i p -> p i"))
        y32 = yp.tile([128, KT, TPC], f32, tag="y32")
        y16 = yp.tile([128, KT, TPC], bf16, tag="y16")
        for i in range(2):
            gth = gthp.tile([128, D], bf16, tag="gth")
            nc.gpsimd.indirect_dma_start(
                out=gth[:], out_offset=None,
                in_=emb_d.ap(),
                in_offset=bass.IndirectOffsetOnAxis(ap=idx[:, i:i + 1], axis=0),
            )
            for j in range(KT):
                tp = psav.tile([128, 256], bf16, tag="av")
                nc.tensor.transpose(out=tp[:, 0:128], in_=gth[:, j * 128:(j + 1) * 128],
                                    identity=ident[:])
                nc.vector.tensor_add(out=y32[:, j, i * 128:(i + 1) * 128],
                                     in0=tp[:, 0:128],
                                     in1=posT[:, j, i * 128:(i + 1) * 128])
        for j in range(KT):
            nc.vector.tensor_copy(out=y16[:, j, :], in_=y32[:, j, :])

        # ---------------- helpers ----------------
        def bias_tile(src_ap, n, tag):
            t = biasp.tile([128, n], f32, tag=tag)
            nc.sync.dma_start(out=t[:], in_=src_ap.rearrange("(m p) -> p m", p=128))
            return t

        def bias_bcast(src_ap, tag):
            row = biasp.tile([1, 512], f32, tag=tag + "r")
            nc.sync.dma_start(out=row[:], in_=src_ap.rearrange("(o c) -> o c", o=1))
            bc = biasp.tile([128, 512], f32, tag=tag)
            nc.gpsimd.partition_broadcast(out_ap=bc[:], in_ap=row[0:1, :])
            return bc

        def mm_lhsw(wdram, l, m, rhs16, kt=KT, wpool=None, wtag=None):
            """psum[128, n] = sum_k W[l,m][:,k,:].T @ rhs16[:,k,:]"""
            wpool = wpool or wt4
            wtile = wpool.tile([128, kt, 128], bf16, tag=wtag or "wt4")
            nc.sync.dma_start(out=wtile[:], in_=wdram.ap()[l, m])
            n = rhs16.shape[-1]
            ps = ps256.tile([128, 256], f32, tag="ps256")
            for k in range(kt):
                nc.tensor.matmul(out=ps[:, 0:n], lhsT=wtile[:, k, :], rhs=rhs16[:, k, :],
                                 start=(k == 0), stop=(k == kt - 1))
            return ps

        def layer_norm(l, i, c32, c16, y32o, y16o):
            """c32 [128,KT,TPC] f32 pre-LN; c16 [128,KT,2*TPC] scratch."""
            for k in range(KT):
                nc.vector.tensor_copy(out=c16[:, k, 0:TPC], in_=c32[:, k, :])
                nc.vector.tensor_mul(out=c16[:, k, TPC:2 * TPC],
                                     in0=c16[:, k, 0:TPC], in1=c16[:, k, 0:TPC])
            stp = psst.tile([1, 512], f32, tag="st")
            for k in range(KT):
                nc.tensor.matmul(out=stp[:], lhsT=ones16[:], rhs=c16[:, k, :],
                                 start=(k == 0), stop=(k == KT - 1))
            st = smp.tile([1, 512], f32, tag="st")
            nc.vector.tensor_scalar_mul(st[:], stp[:], 1.0 / D)
            tmp = smp.tile([1, 256], f32, tag="v1")
            nc.vector.tensor_mul(out=tmp[:], in0=st[0:1, 0:TPC], in1=st[0:1, 0:TPC])
            var = smp.tile([1, 256], f32, tag="v2")
            nc.vector.tensor_sub(out=var[:], in0=st[0:1, TPC:2 * TPC], in1=tmp[:])
            sd = smp.tile([1, 256], f32, tag="v3")
            nc.scalar.activation(out=sd[:], in_=var[:], func=AF.Sqrt, bias=eps_sb[:])
            mr = smp.tile([1, 512], f32, tag="mr")
            nc.vector.reciprocal(out=mr[0:1, TPC:2 * TPC], in_=sd[:])
            nc.vector.tensor_copy(out=mr[0:1, 0:TPC], in_=st[0:1, 0:TPC])
            bc = smp.tile([128, 512], f32, tag="bc512")
            nc.gpsimd.partition_broadcast(out_ap=bc[:], in_ap=mr[0:1, :])
            g = bias_tile(lng_d.ap()[l, i], KT, f"g{i}")
            be = bias_tile(lnb_d.ap()[l, i], KT, f"be{i}")
            for k in range(KT):
                t1 = tmpp.tile([128, 256], f32, tag="t1")
                nc.vector.tensor_sub(out=t1[:], in0=c32[:, k, :], in1=bc[:, 0:TPC])
                t2 = tmpp.tile([128, 256], f32, tag="t2")
                nc.vector.tensor_mul(out=t2[:], in0=t1[:], in1=bc[:, TPC:2 * TPC])
                nc.vector.tensor_scalar(y32o[:, k, :], t2[:], g[:, k:k + 1],
                                        be[:, k:k + 1], mybir.AluOpType.mult,
                                        mybir.AluOpType.add)
                nc.vector.tensor_copy(out=y16o[:, k, :], in_=y32o[:, k, :])

        def attention(qT, kT_fn, v_fn, n_tkt, oT):
            for h in range(H):
                hb = (h % 2) * 64
                pT = ptp.tile([128, NTK, 256], bf16, tag="pT")
                for t in range(n_tkt):
                    sps = ps256.tile([128, 256], f32, tag="ps256")
                    nc.tensor.matmul(out=sps[:], lhsT=kT_fn(h, t),
                                     rhs=qT[hb:hb + 64, h // 2, :],
                                     start=True, stop=True)
                    nc.scalar.activation(out=pT[:, t, :], in_=sps[:], func=AF.Exp,
                                         scale=0.125)
                ops = psav.tile([128, 256], f32, tag="av")
                for t in range(n_tkt):
                    nc.tensor.matmul(out=ops[0:65, :], lhsT=v_fn(h, t), rhs=pT[:, t, :],
                                     start=(t == 0), stop=(t == n_tkt - 1))
                rec = smp.tile([1, 256], f32, tag="rec")
                nc.vector.reciprocal(out=rec[:], in_=ops[64:65, :])
                rbc = smp.tile([64, 256], f32, tag="rbc")
                nc.gpsimd.partition_broadcast(out_ap=rbc[:], in_ap=rec[0:1, :])
                nc.vector.tensor_mul(out=oT[hb:hb + 64, h // 2, :],
                                     in0=ops[0:64, :], in1=rbc[:])

        def out_proj_ln(l, i, wdram, b_sb, oT16, res32, y32o, y16o):
            c32 = cp.tile([128, KT, TPC], f32, tag="c32")
            c16 = cp.tile([128, KT, 2 * TPC], bf16, tag="c16")
            for m in range(KT):
                ps = mm_lhsw(wdram, l, m, oT16)
                t0 = tmpp.tile([128, 256], f32, tag="t0")
                nc.vector.tensor_scalar_add(t0[:], ps[:, 0:TPC], b_sb[:, m:m + 1])
                nc.vector.tensor_add(out=c32[:, m, :], in0=t0[:], in1=res32[:, m, :])
            layer_norm(l, i, c32, c16, y32o, y16o)
            return y32o, y16o

        # ---------------- layers ----------------
        for l in range(L):
            bqkv_sb = bias_tile(bqkv_d.ap()[l, 0:1024], 8, "bqkv")
            bkv_sb = bias_tile(bkv_d.ap()[l, 0:512], 4, "bkvk")
            bq_sb = bias_tile(bq_d.ap()[l], 4, "bq")
            bo_sb = bias_tile(bo_d.ap()[l], 4, "bo")
            bco_sb = bias_tile(bco_d.ap()[l], 4, "bco")
            b1_sb = bias_tile(b1_d.ap()[l], FT, "b1")
            b2_sb = bias_tile(b2_d.ap()[l], 4, "b2")
            vb_bc = bias_bcast(bqkv_d.ap()[l, 1024:1536], "vb")
            vxb_bc = bias_bcast(bkv_d.ap()[l, 512:1024], "vxb")

            # --- self K^T shard -> kv_in[l][0:KV_A] ---
            kT_loc = kvl.tile([128, KT, TPC], bf16, tag="kT_loc")
            for m in range(KT):
                ps = mm_lhsw(wqk_d, l, 4 + m, y16)
                nc.vector.tensor_scalar_add(kT_loc[:, m, :], ps[:, 0:TPC],
                                            bqkv_sb[:, 4 + m:5 + m])
            nc.sync.dma_start(
                out=kv_in[l].ap()[0:KV_A].rearrange("(kt p t) -> p kt t", p=128, t=TPC),
                in_=kT_loc[:])

            # --- self V shard (token-major) -> kv_in[l][KV_A:] ---
            wv = wvr.tile([128, KT, 512], bf16, tag="wvr")
            nc.sync.dma_start(out=wv[:], in_=wqkv_v_d.ap()[l])
            v_loc = kvl.tile([128, 2, D], bf16, tag="v_loc")
            for mt in range(2):
                ps = psw.tile([128, 512], f32, tag="wide")
                for k in range(KT):
                    nc.tensor.matmul(out=ps[:], lhsT=y16[:, k, mt * 128:(mt + 1) * 128],
                                     rhs=wv[:, k, :], start=(k == 0), stop=(k == KT - 1))
                nc.vector.tensor_add(out=v_loc[:, mt, :], in0=ps[:], in1=vb_bc[:])
            nc.sync.dma_start(
                out=kv_in[l].ap()[KV_A:KVN].rearrange("(mt p c) -> p mt c", p=128, c=D),
                in_=v_loc[:])

            # --- AllGather K/V within the 4-core group ---
            nc.gpsimd.collective_compute(
                "AllGather", mybir.AluOpType.bypass, replica_groups=GROUPS,
                ins=[kv_in[l].ap()], outs=[kv_out[l].ap()])

            # --- self q^T (overlaps AG) ---
            qT = qp.tile([128, KT, TPC], bf16, tag="qT")
            for m in range(KT):
                ps = mm_lhsw(wqk_d, l, m, y16)
                nc.vector.tensor_scalar_add(qT[:, m, :], ps[:, 0:TPC],
                                            bqkv_sb[:, m:m + 1])

            # --- cross K^T / V from x (replicated; overlaps AG) ---
            kxT = kxp.tile([128, KT, T], bf16, tag="kxT")
            for m in range(KT):
                wtile = wt4.tile([128, KT, 128], bf16, tag="wt4")
                nc.sync.dma_start(out=wtile[:], in_=wkv_k_d.ap()[l, m])
                for n in range(2):
                    ps = psw.tile([128, 512], f32, tag="wide")
                    for k in range(KT):
                        nc.tensor.matmul(out=ps[:], lhsT=wtile[:, k, :],
                                         rhs=xt16[:, k, n * 512:(n + 1) * 512],
                                         start=(k == 0), stop=(k == KT - 1))
                    nc.vector.tensor_scalar_add(kxT[:, m, n * 512:(n + 1) * 512],
                                                ps[:], bkv_sb[:, m:m + 1])
            wvx = wvr.tile([128, KT, 512], bf16, tag="wvr")
            nc.sync.dma_start(out=wvx[:], in_=wkv_v_d.ap()[l])
            vx = vxp.tile([128, NTK, H, 65], bf16, tag="vx")
            nc.vector.memset(vx[:, :, :, 64:65], 1.0)
            for mt in range(NTK):
                ps = psw.tile([128, 512], f32, tag="wide")
                for k in range(KT):
                    nc.tensor.matmul(out=ps[:], lhsT=xt16[:, k, mt * 128:(mt + 1) * 128],
                                     rhs=wvx[:, k, :], start=(k == 0), stop=(k == KT - 1))
                nc.vector.tensor_add(
                    out=vx[:, mt, :, 0:64],
                    in0=ps[:].rearrange("p (h d) -> p h d", d=64),
                    in1=vxb_bc[:].rearrange("p (h d) -> p h d", d=64))

            # --- load gathered self K/V ---
            kvo4 = kv_out[l].ap().rearrange("(r z) -> r z", r=GP)
            kT_g = khp.tile([128, KT, GP, TPC], bf16, tag="kT_g")
            for j in range(KT):
                nc.sync.dma_start(
                    out=kT_g[:, j, :, :],
                    in_=kvo4[:, 0:KV_A].rearrange("r (kt p t) -> kt p r t", p=128, t=TPC)[j])
            vs = vsp.tile([128, GP, 2, H, 65], bf16, tag="vs")
            nc.vector.memset(vs[:, :, :, :, 64:65], 1.0)
            for r in range(GP):
                for s in range(2):
                    nc.sync.dma_start(
                        out=vs[:, r, s, :, 0:64],
                        in_=kvo4[r, KV_A:KVN].rearrange(
                            "(s p h d) -> s p h d", p=128, h=H, d=HD)[s])

            # --- self attention + out-proj + LN1 ---
            oT = otp.tile([128, KT, TPC], bf16, tag="oT")
            attention(
                qT,
                lambda h, t: kT_g[(h % 2) * 64:(h % 2) * 64 + 64, h // 2,
                                  t // 2, (t % 2) * 128:(t % 2) * 128 + 128],
                lambda h, t: vs[:, t // 2, t % 2, h, :],
                NTK, oT)
            ny32 = yp.tile([128, KT, TPC], f32, tag="y32")
            ny16 = yp.tile([128, KT, TPC], bf16, tag="y16")
            y32, y16 = out_proj_ln(l, 0, wo_d, bo_sb, oT, y32, ny32, ny16)

            # --- cross attention + out-proj + LN2 ---
            qxT = qp.tile([128, KT, TPC], bf16, tag="qT")
            for m in range(KT):
                ps = mm_lhsw(wq_d, l, m, y16)
                nc.vector.tensor_scalar_add(qxT[:, m, :], ps[:, 0:TPC],
                                            bq_sb[:, m:m + 1])
            oxT = otp.tile([128, KT, TPC], bf16, tag="oT")
            attention(
                qxT,
                lambda h, t: kxT[(h % 2) * 64:(h % 2) * 64 + 64, h // 2,
                                 t * 128:(t + 1) * 128],
                lambda h, t: vx[:, t, h, :],
                NTK, oxT)
            ny32 = yp.tile([128, KT, TPC], f32, tag="y32")
            ny16 = yp.tile([128, KT, TPC], bf16, tag="y16")
            y32, y16 = out_proj_ln(l, 1, wco_d, bco_sb, oxT, y32, ny32, ny16)

            # --- FFN + LN3 ---
            hT = htp.tile([128, FT, TPC], bf16, tag="hT")
            for m in range(FT):
                ps = mm_lhsw(w1_d, l, m, y16)
                nc.scalar.activation(out=hT[:, m, :], in_=ps[:, 0:TPC], func=AF.Relu,
                                     bias=b1_sb[:, m:m + 1])
            c32 = cp.tile([128, KT, TPC], f32, tag="c32")
            c16 = cp.tile([128, KT, 2 * TPC], bf16, tag="c16")
            for m in range(KT):
                ps = mm_lhsw(w2_d, l, m, hT, kt=FT, wpool=wt16, wtag="wt16")
                t0 = tmpp.tile([128, 256], f32, tag="t0")
                nc.vector.tensor_scalar_add(t0[:], ps[:, 0:TPC], b2_sb[:, m:m + 1])
                nc.vector.tensor_add(out=c32[:, m, :], in0=t0[:], in1=y32[:, m, :])
            ny32 = yp.tile([128, KT, TPC], f32, tag="y32")
            ny16 = yp.tile([128, KT, TPC], bf16, tag="y16")
            layer_norm(l, 2, c32, c16, ny32, ny16)
            y32, y16 = ny32, ny16

        nc.sync.dma_start(out=out_d.ap().rearrange("(kt p) t -> p kt t", p=128), in_=y32[:])

    nc.compile()
    return nc


# --------------------------------------------------------------------------
# host side
# --------------------------------------------------------------------------

def _pos_enc():
    even = np.arange(0, D, 2, dtype=np.float32)
    denom = np.power(np.float32(10000.0), even / np.float32(D))
    pos = np.arange(T, dtype=np.float32)[:, None]
    return np.stack([np.sin(pos / denom), np.cos(pos / denom)], axis=2).reshape(T, D)


def _tile_w(w):
    """[K, C] -> [C//128, 128, K//128, 128] with out[m,p,kt,c] = w[kt*128+p, m*128+c]"""
    K_, C_ = w.shape
    return np.ascontiguousarray(
        w.reshape(K_ // 128, 128, C_ // 128, 128).transpose(2, 1, 0, 3)).astype(BF)


def _tile_w_rhs(w):
    """[K, C] -> [128, K//128, C] with out[p,kt,c] = w[kt*128+p, c]"""
    K_, C_ = w.shape
    return np.ascontiguousarray(
        w.reshape(K_ // 128, 128, C_).transpose(1, 0, 2)).astype(BF)


def _numpy_reference(x, y_tokens, self_mask, cross_mask, emb, Wqkv, bqkv, Wo, bo,
                     Wkv, bkv, Wq, bq, Wco, bco, W1, b1, W2, b2,
                     g1, be1, g2, be2, g3, be3):
    def ln(v, g, b):
        m = v.mean(-1, keepdims=True)
        s = v.var(-1, keepdims=True)
        return (v - m) / np.sqrt(s + EPS) * g + b

    def heads(t):
        return t.reshape(B, T, H, HD).transpose(0, 2, 1, 3)

    def attn(q, k, v, mask):
        s = np.einsum('bhqd,bhkd->bhqk', q, k) / np.sqrt(HD).astype(np.float32)
        s = s + mask[:, None]
        s = s - s.max(-1, keepdims=True)
        a = np.exp(s)
        a /= a.sum(-1, keepdims=True)
        o = np.einsum('bhqk,bhkd->bhqd', a, v)
        return o.transpose(0, 2, 1, 3).reshape(B, T, D)

    y = emb[y_tokens] + _pos_enc()
    for l in range(L):
        r = y
        q, k, v = np.split(y @ Wqkv[l] + bqkv[l], 3, axis=-1)
        a = attn(heads(q), heads(k), heads(v), self_mask)
        y = ln(a @ Wo[l] + bo[l] + r, g1[l], be1[l])
        r = y
        k, v = np.split(x @ Wkv[l] + bkv[l], 2, axis=-1)
        q = y @ Wq[l] + bq[l]
        a = attn(heads(q), heads(k), heads(v), cross_mask)
        y = ln(a @ Wco[l] + bco[l] + r, g2[l], be2[l])
        r = y
        hh = np.maximum(y @ W1[l] + b1[l], 0.0)
        y = ln(hh @ W2[l] + b2[l] + r, g3[l], be3[l])
    return y.astype(np.float32)


def _prep_in_maps(inputs):
    np32 = lambda a: np.asarray(a, dtype=np.float32)
    x = np32(inputs["x"])
    y_tokens = np.asarray(inputs["y_tokens"], dtype=np.int32)
    emb16 = np32(inputs["emb"]).astype(BF)
    pe = _pos_enc()

    Wqkv = np32(inputs["Wqkv"])
    shared = dict(
        emb16=emb16,
        wqk=np.stack([_tile_w(Wqkv[l][:, 0:1024]) for l in range(L)]),
        wqkv_v=np.stack([_tile_w_rhs(Wqkv[l][:, 1024:1536]) for l in range(L)]),
        wo=np.stack([_tile_w(np32(inputs["Wo"])[l]) for l in range(L)]),
        wkv_k=np.stack([_tile_w(np32(inputs["Wkv"])[l][:, 0:512]) for l in range(L)]),
        wkv_v=np.stack([_tile_w_rhs(np32(inputs["Wkv"])[l][:, 512:1024]) for l in range(L)]),
        wq=np.stack([_tile_w(np32(inputs["Wq"])[l]) for l in range(L)]),
        wco=np.stack([_tile_w(np32(inputs["Wco"])[l]) for l in range(L)]),
        w1=np.stack([_tile_w(np32(inputs["W1"])[l]) for l in range(L)]),
        w2=np.stack([_tile_w(np32(inputs["W2"])[l]) for l in range(L)]),
        bqkv=np32(inputs["bqkv"]), bkv=np32(inputs["bkv"]),
        bq=np32(inputs["bq"]), bo=np32(inputs["bo"]), bco=np32(inputs["bco"]),
        b1=np32(inputs["b1"]), b2=np32(inputs["b2"]),
        lng=np.stack([np32(inputs["g1"]), np32(inputs["g2"]), np32(inputs["g3"])], axis=1),
        lnb=np.stack([np32(inputs["be1"]), np32(inputs["be2"]), np32(inputs["be3"])], axis=1),
    )

    in_maps = []
    for c in range(NC):
        b, r = c // GP, c % GP
        sl = slice(r * TPC, (r + 1) * TPC)
        m = dict(shared)
        m["tok"] = np.ascontiguousarray(y_tokens[b, sl].reshape(2, 128))
        m["posT"] = np.ascontiguousarray(pe[sl].T)
        m["xT16"] = np.ascontiguousarray(x[b].T).astype(BF)
        in_maps.append(m)
    return in_maps


def kernel(**inputs) -> np.ndarray:
    if np.any(np.asarray(inputs["self_mask"])) or np.any(np.asarray(inputs["cross_mask"])):
        args = {k: np.asarray(v, dtype=(np.int32 if k == "y_tokens" else np.float32))
                for k, v in inputs.items()}
        return _numpy_reference(**args)

    if "nc" not in _CACHE:
        _CACHE["nc"] = _build_module()
    nc = _CACHE["nc"]

    in_maps = _prep_in_maps(inputs)
    res = run_bass_kernel_spmd(nc, in_maps, core_ids=list(range(NC)))

    out = np.empty((B, T, D), dtype=np.float32)
    for c in range(NC):
        b, r = c // GP, c % GP
        out[b, r * TPC:(r + 1) * TPC, :] = res.results[c]["yT_out"].T
    return out
